# revision 15
# baseline (speedup 1.0000x reference)
"""Trainium2 SPMD kernel for: y = BatchNorm1d(x @ sign(w).T + bias) * gamma + beta.

Sharding: data-parallel over the batch dim across 8 NeuronCores; the
weight is replicated.  BatchNorm batch statistics are produced with
on-device AllReduces of per-shard (sum_y, sum_y2).

Layout: weight-stationary matmul producing y in [o, b] layout (output
features on partitions).  This makes the batch reductions free-dim
reductions (fused into the PSUM->SBUF copy via accum_out on DVE and a
Square pass on ACT), removes all stats matmuls from the tensor engine,
and turns the BN normalize into a single per-partition scale+bias op.

Pipelining:
  - The first pass computes output chunks 0 and 1 jointly, so the PE
    consumes x stripes at half rate while they stream from HBM (no
    input stall); later passes run one chunk at full rate on resident x.
  - The stats AllReduce is split in three: A(oc0..3) and B(oc4..6) run
    and normalize *during* later matmul passes; only the tiny C(oc7)
    AllReduce and one normalize+store chunk sit after the final matmul.

Math notes:
  - The linear bias cancels inside BatchNorm (y - mean), so it is never
    applied on device.
  - sign(w) in {-1,+1} is realized as +-0.5 via integer ops on the bf16
    bit pattern: (w & 0x8000) | 0x3F00.  The global scale of 0.5 cancels
    in BatchNorm except in the epsilon, which is compensated with eps/4.
    (A float is_ge on bf16 input hits a slow DVE microcode path; the
    bitwise form runs at full rate.)
  - Matmul runs in bf16 (weights +-0.5 exact; x rounding ~2e-3 rel err).
    x and w are cast to bf16 on the host (same numerics as a casting
    DMA, half the HBM traffic).  The output is written bf16 and upcast
    on the host (the values are bf16-rounded before the store either
    way, so this is numerically identical to a casting store DMA).
"""

import os
import sys

sys.path.insert(0, "/opt/trn_rl_repo")

import numpy as np

import concourse.bacc as bacc
import concourse.mybir as mybir
import concourse.tile as tile
from concourse import bass_utils

N_CORES = 8
B_TOT = 16384
D_IN = 2048
D_OUT = 1024
B_SH = B_TOT // N_CORES          # 2048 rows per core
KT = D_IN // 128                 # 16 contraction stripes
OC = D_OUT // 128                # 8 output chunks (partition dim of y)
NB = 512                         # moving free dim per MM (PSUM bank limit)
BC = B_SH // NB                  # 4 batch chunks per core
BN_EPS = 1e-5

F32 = mybir.dt.float32
BF16 = mybir.dt.bfloat16
U16 = mybir.dt.uint16

# oc passes after which a dummy warm-up all-reduce fires ("" = none; the
# CC firmware init autostarts at NEFF load when collectives are present)
WARM_OCS = tuple(
    int(s) for s in os.environ.get("KERNEL_WARM_OCS", "").split(",") if s != ""
)


def build_kernel():
    nc = bacc.Bacc("TRN2", target_bir_lowering=False, debug=False,
                   num_devices=N_CORES)

    # x^T per shard: [k, b], contiguous 4KB rows
    xt = nc.dram_tensor("xt", [D_IN, B_SH], BF16, kind="ExternalInput")
    # w^T: [k, o], contiguous 2KB rows
    wt = nc.dram_tensor("wt", [D_IN, D_OUT], BF16, kind="ExternalInput")
    # gamma/beta pre-laid-out as [128, OC] (partition p = o % 128, col oc)
    gamma = nc.dram_tensor("gamma", [128, OC], F32, kind="ExternalInput")
    beta = nc.dram_tensor("beta", [128, OC], F32, kind="ExternalInput")
    # output in device layout [oc*128 + p, b]; host transposes back
    out = nc.dram_tensor("out", [D_OUT, B_SH], BF16, kind="ExternalOutput")

    with tile.TileContext(nc) as tc:
        with tc.tile_pool(name="persist", bufs=1) as persist, \
             tc.tile_pool(name="work", bufs=2) as work_pool, \
             tc.tile_pool(name="stage", bufs=3) as stage_pool, \
             tc.tile_pool(name="psum", bufs=2, space="PSUM") as psum_pool, \
             tc.tile_pool(name="dram", bufs=1, space="DRAM") as dram:

            # ---- per-stripe loads, balanced across both HWDGE rings;
            # ---- binarize w to {-0.5,+0.5} with integer bit ops on DVE
            wbs = []
            xs = []
            for it in range(KT):
                wraw = work_pool.tile([128, D_OUT], BF16, name=f"wraw{it}",
                                      tag=f"wraw{it % 4}")
                weng = nc.sync if it % 2 == 0 else nc.scalar
                weng.dma_start(wraw[:], wt[it * 128:(it + 1) * 128, :])
                wb = persist.tile([128, D_OUT], BF16, name=f"wb{it}")
                # (w & 0x8000) | 0x3F00  ->  +-0.5 in bf16
                nc.vector.tensor_scalar(
                    out=wb.bitcast(U16)[:], in0=wraw.bitcast(U16)[:],
                    scalar1=0x8000, scalar2=0x3F00,
                    op0=mybir.AluOpType.bitwise_and,
                    op1=mybir.AluOpType.bitwise_or,
                )
                wbs.append(wb)

                xtile = persist.tile([128, B_SH], BF16, name=f"xs{it}")
                xeng = nc.scalar if it % 2 == 0 else nc.sync
                xeng.dma_start(xtile[:], xt[it * 128:(it + 1) * 128, :])
                xs.append(xtile)

            # ---- persistent y (bf16, [o, b] layout) and per-group stats ----
            y_all = persist.tile([128, OC * B_SH], BF16)   # [p, (oc, b)]
            # AR groups: A = oc0..3, B = oc4..7 (the CC engine runs ARs
            # serially at ~16-22us each, so only one may sit on the tail)
            GROUPS = [(0, 1, 2, 3), (4, 5, 6, 7)]
            OC_GROUP = {oc: g for g, ocs in enumerate(GROUPS) for oc in ocs}
            sacc = [persist.tile([128, 2 * len(ocs)], F32, name=f"sacc{g}")
                    for g, ocs in enumerate(GROUPS)]

            # gamma/beta in [128, OC] layout (tiny; off the load rings)
            gam = persist.tile([128, OC], F32)
            bet = persist.tile([128, OC], F32)
            nc.gpsimd.dma_start(gam[:], gamma[:, :])
            nc.gpsimd.dma_start(bet[:], beta[:, :])

            # coefficient tiles (written per group, read per oc chunk)
            a_c = persist.tile([128, OC], F32)
            c_c = persist.tile([128, OC], F32)
            mean = persist.tile([128, OC], F32)
            var = persist.tile([128, OC], F32)
            inv = persist.tile([128, OC], F32)
            m2 = persist.tile([128, OC], F32)
            nma = persist.tile([128, OC], F32)

            eps_t = persist.tile([128, 1], F32)
            nc.vector.memset(eps_t[:], BN_EPS / 4.0)

            # AR payload in bf16: halves the (payload-sensitive) collective
            # latency; the stats sums lose ~2^-8 relative, ~0.2% on sigma
            sbf = [persist.tile([128, 2 * len(ocs)], BF16, name=f"sbf{g}")
                   for g, ocs in enumerate(GROUPS)]
            cbis = [dram.tile([128, 2 * len(ocs)], BF16, name=f"cbi{g}",
                              tag=f"cbi{g}") for g, ocs in enumerate(GROUPS)]
            cbos = [dram.tile([128, 2 * len(ocs)], BF16, name=f"cbo{g}",
                              tag=f"cbo{g}") for g, ocs in enumerate(GROUPS)]
            gss = [persist.tile([128, 2 * len(ocs)], BF16, name=f"gs{g}")
                   for g, ocs in enumerate(GROUPS)]

            def do_group_ar(g):
                """DMA stats of group g out, all-reduce, read back."""
                nc.vector.tensor_copy(sbf[g][:], sacc[g][:])
                nc.sync.dma_start(cbis[g][:], sbf[g][:])
                nc.gpsimd.collective_compute(
                    "AllReduce", mybir.AluOpType.add,
                    replica_groups=[list(range(N_CORES))],
                    ins=[cbis[g].opt()], outs=[cbos[g].opt()],
                )
                nc.sync.dma_start(gss[g][:], cbos[g][:])

            def do_group_coef(g):
                """a = gamma/sqrt(var+eps/4), c = beta - mean*a for group g."""
                ocs = GROUPS[g]
                n = len(ocs)
                lo, hi = ocs[0], ocs[0] + n
                gs = gss[g]
                nc.vector.tensor_scalar_mul(mean[:, lo:hi], gs[:, 0:n],
                                            1.0 / B_TOT)
                nc.vector.tensor_tensor(out=m2[:, lo:hi], in0=mean[:, lo:hi],
                                        in1=mean[:, lo:hi],
                                        op=mybir.AluOpType.mult)
                # var = E[y^2]/B - mean^2
                nc.vector.scalar_tensor_tensor(
                    out=var[:, lo:hi], in0=gs[:, n:2 * n],
                    scalar=1.0 / B_TOT, in1=m2[:, lo:hi],
                    op0=mybir.AluOpType.mult,
                    op1=mybir.AluOpType.subtract)
                # sqrt(var*1 + eps/4): eps folds into the activation bias
                nc.scalar.activation(inv[:, lo:hi], var[:, lo:hi],
                                     mybir.ActivationFunctionType.Sqrt,
                                     bias=eps_t[:, 0:1])
                nc.vector.reciprocal(inv[:, lo:hi], inv[:, lo:hi])
                nc.vector.tensor_tensor(out=a_c[:, lo:hi], in0=gam[:, lo:hi],
                                        in1=inv[:, lo:hi],
                                        op=mybir.AluOpType.mult)
                # c = beta - mean*a  via  nma = (mean * -1) * a ; c = beta+nma
                nc.vector.scalar_tensor_tensor(
                    out=nma[:, lo:hi], in0=mean[:, lo:hi], scalar=-1.0,
                    in1=a_c[:, lo:hi],
                    op0=mybir.AluOpType.mult, op1=mybir.AluOpType.mult)
                nc.vector.tensor_tensor(out=c_c[:, lo:hi], in0=bet[:, lo:hi],
                                        in1=nma[:, lo:hi],
                                        op=mybir.AluOpType.add)

            def do_norm_store(oc, on_vector):
                """Normalize y chunk oc with per-partition a,c and store."""
                stg = stage_pool.tile([128, B_SH], BF16, name=f"stg{oc}",
                                      tag="stg")
                ysl = y_all[:, oc * B_SH:(oc + 1) * B_SH]
                if on_vector:
                    nc.vector.tensor_scalar(
                        out=stg[:], in0=ysl,
                        scalar1=a_c[:, oc:oc + 1], scalar2=c_c[:, oc:oc + 1],
                        op0=mybir.AluOpType.mult,
                        op1=mybir.AluOpType.add,
                    )
                else:
                    nc.scalar.activation(
                        stg[:], ysl,
                        mybir.ActivationFunctionType.Identity,
                        bias=c_c[:, oc:oc + 1], scale=a_c[:, oc:oc + 1],
                    )
                eng = nc.sync if oc % 2 == 0 else nc.scalar
                eng.dma_start(out[oc * 128:(oc + 1) * 128, :], stg[:])

            def do_stats(oc, pt):
                """Evacuate PSUM for chunk oc: DVE copies+sums y into y_all,
                ACT squares+sums from the bf16 copy."""
                g = OC_GROUP[oc]
                ocs = GROUPS[g]
                n = len(ocs)
                j = oc - ocs[0]
                py = work_pool.tile([128, 4], F32, name=f"py{oc}",
                                    tag=f"py{oc % 2}")
                py2 = work_pool.tile([128, 4], F32, name=f"py2{oc}",
                                     tag=f"py2{oc % 2}")
                for q in range(4):
                    nc.vector.tensor_scalar(
                        out=y_all[:, oc * B_SH + q * 512:
                                  oc * B_SH + q * 512 + 512],
                        in0=pt[:, q * 512:q * 512 + 512],
                        scalar1=1.0, scalar2=None,
                        op0=mybir.AluOpType.mult,
                        op1=mybir.AluOpType.add,
                        accum_out=py[:, q:q + 1],
                    )
                for q in range(4):
                    y2scr = work_pool.tile([128, 512], BF16,
                                           name=f"y2_{oc}_{q}",
                                           tag=f"y2_{q % 2}")
                    nc.scalar.activation(
                        y2scr[:],
                        y_all[:, oc * B_SH + q * 512:oc * B_SH + q * 512 + 512],
                        mybir.ActivationFunctionType.Square,
                        accum_out=py2[:, q:q + 1],
                    )
                nc.vector.reduce_sum(out=sacc[g][:, j:j + 1], in_=py[:],
                                     axis=mybir.AxisListType.X)
                nc.vector.reduce_sum(out=sacc[g][:, n + j:n + j + 1],
                                     in_=py2[:], axis=mybir.AxisListType.X)

            def do_warm(oc):
                wsrc = work_pool.tile([1, 8], F32, name=f"wsrc{oc}",
                                      tag="wsrc")
                nc.vector.memset(wsrc[:], 1.0)
                wi = dram.tile([1, 8], F32, name=f"warm_i{oc}",
                               tag=f"warm_i{oc}")
                wo = dram.tile([1, 8], F32, name=f"warm_o{oc}",
                               tag=f"warm_o{oc}")
                nc.gpsimd.dma_start(wi[:], wsrc[:])
                nc.gpsimd.collective_compute(
                    "AllReduce", mybir.AluOpType.add,
                    replica_groups=[list(range(N_CORES))],
                    ins=[wi.opt()], outs=[wo.opt()],
                )

            # ---- joint first pass: oc0 + oc1 while x streams in ----
            pts = {}
            pts[0] = psum_pool.tile([128, B_SH], F32, name="pt0", tag="pt")
            pts[1] = psum_pool.tile([128, B_SH], F32, name="pt1", tag="pt")
            for it in range(KT):
                for oc in (0, 1):
                    for bc in range(BC):
                        nc.tensor.matmul(
                            pts[oc][:, bc * NB:bc * NB + NB],
                            wbs[it][:, oc * 128:oc * 128 + 128],
                            xs[it][:, bc * NB:bc * NB + NB],
                            start=(it == 0), stop=(it == KT - 1),
                        )
            do_stats(0, pts[0])
            do_stats(1, pts[1])
            if 0 in WARM_OCS:
                do_warm(0)

            # ---- remaining passes: one oc each ----
            for oc in range(2, OC):
                pt = psum_pool.tile([128, B_SH], F32, name=f"pt{oc}",
                                    tag="pt")
                for it in range(KT):
                    for bc in range(BC):
                        nc.tensor.matmul(
                            pt[:, bc * NB:bc * NB + NB],
                            wbs[it][:, oc * 128:oc * 128 + 128],
                            xs[it][:, bc * NB:bc * NB + NB],
                            start=(it == 0), stop=(it == KT - 1),
                        )
                do_stats(oc, pt)

                if oc in WARM_OCS:
                    do_warm(oc)
                if oc == 3:
                    do_group_ar(0)       # overlaps oc4..7 matmuls
                if oc == 5:
                    do_group_coef(0)
                if oc == 6:
                    do_norm_store(0, on_vector=True)
                    do_norm_store(1, on_vector=False)
                    do_norm_store(2, on_vector=True)
                    do_norm_store(3, on_vector=True)

            # ---- tail: group B AR + its normalize/store ----
            do_group_ar(1)
            do_group_coef(1)
            do_norm_store(4, on_vector=True)
            do_norm_store(5, on_vector=False)
            do_norm_store(6, on_vector=True)
            do_norm_store(7, on_vector=True)

    nc.compile()
    return nc


_NC_CACHE = None


def kernel(x, weight, bias, gamma, beta):
    global _NC_CACHE
    if _NC_CACHE is None:
        _NC_CACHE = build_kernel()
    nc = _NC_CACHE

    import ml_dtypes
    bf16 = ml_dtypes.bfloat16

    x = np.asarray(x, dtype=np.float32)
    weight = np.asarray(weight, dtype=np.float32)
    # gamma/beta -> [128, OC] with partition p = o % 128, column oc
    gamma_t = np.ascontiguousarray(
        np.asarray(gamma, dtype=np.float32).reshape(OC, 128).T)
    beta_t = np.ascontiguousarray(
        np.asarray(beta, dtype=np.float32).reshape(OC, 128).T)

    wt = np.ascontiguousarray(weight.T).astype(bf16)
    in_maps = []
    for i in range(N_CORES):
        shard = x[i * B_SH:(i + 1) * B_SH]
        in_maps.append({
            "xt": np.ascontiguousarray(shard.T).astype(bf16),
            "wt": wt,
            "gamma": gamma_t,
            "beta": beta_t,
        })

    res = bass_utils.run_bass_kernel_spmd(
        nc, in_maps, core_ids=list(range(N_CORES)),
        trace=bool(int(os.environ.get("KERNEL_TRACE", "0"))),
    )
    kernel.last_results = res
    # device output is [o, b] bf16; transpose back and upcast
    return np.concatenate(
        [np.asarray(res.results[i]["out"]).T.astype(np.float32)
         for i in range(N_CORES)], axis=0)


# revision 21
# speedup vs baseline: 1.0213x; 1.0213x over previous
"""Trainium2 SPMD kernel for: y = BatchNorm1d(x @ sign(w).T + bias) * gamma + beta.

Sharding: data-parallel over the batch dim across 8 NeuronCores; the
weight is replicated.  BatchNorm batch statistics are produced with
on-device AllReduces of per-shard (sum_y, sum_y2).

Layout: weight-stationary matmul producing y in [o, b] layout (output
features on partitions).  This makes the batch reductions free-dim
reductions (fused into the PSUM->SBUF copy via accum_out on DVE and a
Square pass on ACT), removes all stats matmuls from the tensor engine,
and turns the BN normalize into a single per-partition scale+bias op.

Pipelining:
  - The first pass computes output chunks 0 and 1 jointly, so the PE
    consumes x stripes at half rate while they stream from HBM (no
    input stall); later passes run one chunk at full rate on resident x.
  - The stats AllReduce is split in three: A(oc0..3) and B(oc4..6) run
    and normalize *during* later matmul passes; only the tiny C(oc7)
    AllReduce and one normalize+store chunk sit after the final matmul.

Math notes:
  - The linear bias cancels inside BatchNorm (y - mean), so it is never
    applied on device.
  - sign(w) in {-1,+1} is realized as +-0.5 via integer ops on the bf16
    bit pattern: (w & 0x8000) | 0x3F00.  The global scale of 0.5 cancels
    in BatchNorm except in the epsilon, which is compensated with eps/4.
    (A float is_ge on bf16 input hits a slow DVE microcode path; the
    bitwise form runs at full rate.)
  - Matmul runs in bf16 (weights +-0.5 exact; x rounding ~2e-3 rel err).
    x and w are cast to bf16 on the host (same numerics as a casting
    DMA, half the HBM traffic).  The output is written bf16 and upcast
    on the host (the values are bf16-rounded before the store either
    way, so this is numerically identical to a casting store DMA).
"""

import os
import sys

sys.path.insert(0, "/opt/trn_rl_repo")

import numpy as np

import concourse.bacc as bacc
import concourse.mybir as mybir
import concourse.tile as tile
from concourse import bass_utils

N_CORES = 8
B_TOT = 16384
D_IN = 2048
D_OUT = 1024
B_SH = B_TOT // N_CORES          # 2048 rows per core
KT = D_IN // 128                 # 16 contraction stripes
OC = D_OUT // 128                # 8 output chunks (partition dim of y)
NB = 512                         # moving free dim per MM (PSUM bank limit)
BC = B_SH // NB                  # 4 batch chunks per core
BN_EPS = 1e-5

F32 = mybir.dt.float32
BF16 = mybir.dt.bfloat16
U16 = mybir.dt.uint16

# oc passes after which a dummy warm-up all-reduce fires ("" = none; the
# CC firmware init autostarts at NEFF load when collectives are present)
WARM_OCS = tuple(
    int(s) for s in os.environ.get("KERNEL_WARM_OCS", "").split(",") if s != ""
)


def build_kernel():
    nc = bacc.Bacc("TRN2", target_bir_lowering=False, debug=False,
                   num_devices=N_CORES)

    # x^T per shard: [k, b], contiguous 4KB rows
    xt = nc.dram_tensor("xt", [D_IN, B_SH], BF16, kind="ExternalInput")
    # w^T: [k, o], contiguous 2KB rows
    wt = nc.dram_tensor("wt", [D_IN, D_OUT], BF16, kind="ExternalInput")
    # gamma/beta pre-laid-out as [128, OC] (partition p = o % 128, col oc)
    gamma = nc.dram_tensor("gamma", [128, OC], F32, kind="ExternalInput")
    beta = nc.dram_tensor("beta", [128, OC], F32, kind="ExternalInput")
    # output in device layout [oc*128 + p, b]; host transposes back
    out = nc.dram_tensor("out", [D_OUT, B_SH], BF16, kind="ExternalOutput")

    with tile.TileContext(nc) as tc:
        with tc.tile_pool(name="persist", bufs=1) as persist, \
             tc.tile_pool(name="work", bufs=2) as work_pool, \
             tc.tile_pool(name="stage", bufs=3) as stage_pool, \
             tc.tile_pool(name="psum", bufs=2, space="PSUM") as psum_pool, \
             tc.tile_pool(name="dram", bufs=1, space="DRAM") as dram:

            # ---- per-stripe loads, balanced across both HWDGE rings;
            # ---- binarize w to {-0.5,+0.5} with integer bit ops on DVE
            wbs = []
            xs = []
            for it in range(KT):
                wraw = work_pool.tile([128, D_OUT], BF16, name=f"wraw{it}",
                                      tag=f"wraw{it % 4}")
                weng = nc.sync if it % 2 == 0 else nc.scalar
                weng.dma_start(wraw[:], wt[it * 128:(it + 1) * 128, :])
                wb = persist.tile([128, D_OUT], BF16, name=f"wb{it}")
                # (w & 0x8000) | 0x3F00  ->  +-0.5 in bf16
                nc.vector.tensor_scalar(
                    out=wb.bitcast(U16)[:], in0=wraw.bitcast(U16)[:],
                    scalar1=0x8000, scalar2=0x3F00,
                    op0=mybir.AluOpType.bitwise_and,
                    op1=mybir.AluOpType.bitwise_or,
                )
                wbs.append(wb)

                xtile = persist.tile([128, B_SH], BF16, name=f"xs{it}")
                if it == 0:
                    # quarter the first stripe so the first matmul's input
                    # lands ~1.5us earlier
                    for q in range(4):
                        qeng = nc.scalar if q % 2 == 0 else nc.sync
                        qeng.dma_start(
                            xtile[:, q * 512:q * 512 + 512],
                            xt[0:128, q * 512:q * 512 + 512])
                else:
                    xeng = nc.scalar if it % 2 == 0 else nc.sync
                    xeng.dma_start(xtile[:], xt[it * 128:(it + 1) * 128, :])
                xs.append(xtile)

            # ---- persistent y (bf16, [o, b] layout) and per-group stats ----
            y_all = persist.tile([128, OC * B_SH], BF16)   # [p, (oc, b)]
            # AR groups: A = oc0..3, B = oc4..7 (the CC engine runs ARs
            # serially at ~16-22us each, so only one may sit on the tail)
            GROUPS = [(0, 1, 2, 3), (4, 5, 6, 7)]
            OC_GROUP = {oc: g for g, ocs in enumerate(GROUPS) for oc in ocs}
            sacc = [persist.tile([128, 2 * len(ocs)], F32, name=f"sacc{g}")
                    for g, ocs in enumerate(GROUPS)]

            # gamma/beta in [128, OC] layout (tiny; off the load rings)
            gam = persist.tile([128, OC], F32)
            bet = persist.tile([128, OC], F32)
            nc.gpsimd.dma_start(gam[:], gamma[:, :])
            nc.gpsimd.dma_start(bet[:], beta[:, :])

            # coefficient tiles (written per group, read per oc chunk)
            a_c = persist.tile([128, OC], F32)
            c_c = persist.tile([128, OC], F32)
            mean = persist.tile([128, OC], F32)
            var = persist.tile([128, OC], F32)
            inv = persist.tile([128, OC], F32)
            m2 = persist.tile([128, OC], F32)
            nma = persist.tile([128, OC], F32)

            eps_t = persist.tile([128, 1], F32)
            nc.vector.memset(eps_t[:], BN_EPS / 4.0)

            cbis = [dram.tile([128, 2 * len(ocs)], F32, name=f"cbi{g}",
                              tag=f"cbi{g}") for g, ocs in enumerate(GROUPS)]
            cbos = [dram.tile([128, 2 * len(ocs)], F32, name=f"cbo{g}",
                              tag=f"cbo{g}") for g, ocs in enumerate(GROUPS)]
            gss = [persist.tile([128, 2 * len(ocs)], F32, name=f"gs{g}")
                   for g, ocs in enumerate(GROUPS)]

            def do_group_ar(g):
                """DMA stats of group g out, all-reduce, read back."""
                nc.sync.dma_start(cbis[g][:], sacc[g][:])
                nc.gpsimd.collective_compute(
                    "AllReduce", mybir.AluOpType.add,
                    replica_groups=[list(range(N_CORES))],
                    ins=[cbis[g].opt()], outs=[cbos[g].opt()],
                )
                nc.sync.dma_start(gss[g][:], cbos[g][:])

            def do_group_coef(g):
                """a = gamma/sqrt(var+eps/4), c = beta - mean*a for group g."""
                ocs = GROUPS[g]
                n = len(ocs)
                lo, hi = ocs[0], ocs[0] + n
                gs = gss[g]
                nc.vector.tensor_scalar_mul(mean[:, lo:hi], gs[:, 0:n],
                                            1.0 / B_TOT)
                nc.vector.tensor_tensor(out=m2[:, lo:hi], in0=mean[:, lo:hi],
                                        in1=mean[:, lo:hi],
                                        op=mybir.AluOpType.mult)
                # var = E[y^2]/B - mean^2
                nc.vector.scalar_tensor_tensor(
                    out=var[:, lo:hi], in0=gs[:, n:2 * n],
                    scalar=1.0 / B_TOT, in1=m2[:, lo:hi],
                    op0=mybir.AluOpType.mult,
                    op1=mybir.AluOpType.subtract)
                # sqrt(var*1 + eps/4): eps folds into the activation bias
                nc.scalar.activation(inv[:, lo:hi], var[:, lo:hi],
                                     mybir.ActivationFunctionType.Sqrt,
                                     bias=eps_t[:, 0:1])
                nc.vector.reciprocal(inv[:, lo:hi], inv[:, lo:hi])
                nc.vector.tensor_tensor(out=a_c[:, lo:hi], in0=gam[:, lo:hi],
                                        in1=inv[:, lo:hi],
                                        op=mybir.AluOpType.mult)
                # c = beta - mean*a  via  nma = (mean * -1) * a ; c = beta+nma
                nc.vector.scalar_tensor_tensor(
                    out=nma[:, lo:hi], in0=mean[:, lo:hi], scalar=-1.0,
                    in1=a_c[:, lo:hi],
                    op0=mybir.AluOpType.mult, op1=mybir.AluOpType.mult)
                nc.vector.tensor_tensor(out=c_c[:, lo:hi], in0=bet[:, lo:hi],
                                        in1=nma[:, lo:hi],
                                        op=mybir.AluOpType.add)

            def do_norm_store(oc, on_vector):
                """Normalize y chunk oc with per-partition a,c and store."""
                stg = stage_pool.tile([128, B_SH], BF16, name=f"stg{oc}",
                                      tag="stg")
                ysl = y_all[:, oc * B_SH:(oc + 1) * B_SH]
                if on_vector:
                    nc.vector.tensor_scalar(
                        out=stg[:], in0=ysl,
                        scalar1=a_c[:, oc:oc + 1], scalar2=c_c[:, oc:oc + 1],
                        op0=mybir.AluOpType.mult,
                        op1=mybir.AluOpType.add,
                    )
                else:
                    nc.scalar.activation(
                        stg[:], ysl,
                        mybir.ActivationFunctionType.Identity,
                        bias=c_c[:, oc:oc + 1], scale=a_c[:, oc:oc + 1],
                    )
                eng = nc.sync if oc % 2 == 0 else nc.scalar
                eng.dma_start(out[oc * 128:(oc + 1) * 128, :], stg[:])

            def do_stats(oc, pt):
                """Evacuate PSUM for chunk oc: DVE copies+sums y into y_all,
                ACT squares+sums from the bf16 copy."""
                g = OC_GROUP[oc]
                ocs = GROUPS[g]
                n = len(ocs)
                j = oc - ocs[0]
                py = work_pool.tile([128, 4], F32, name=f"py{oc}",
                                    tag=f"py{oc % 2}")
                py2 = work_pool.tile([128, 4], F32, name=f"py2{oc}",
                                     tag=f"py2{oc % 2}")

                def ysl(q):
                    return y_all[:, oc * B_SH + q * 512:
                                 oc * B_SH + q * 512 + 512]

                # DVE copies+sums y out of PSUM; ACT squares+sums from the
                # bf16 copy (keeps PSUM single-reader, engines pipeline)
                for q in range(4):
                    nc.vector.tensor_scalar(
                        out=ysl(q), in0=pt[:, q * 512:q * 512 + 512],
                        scalar1=1.0, scalar2=None,
                        op0=mybir.AluOpType.mult,
                        op1=mybir.AluOpType.add,
                        accum_out=py[:, q:q + 1],
                    )
                for q in range(4):
                    y2scr = work_pool.tile([128, 512], BF16,
                                           name=f"y2_{oc}_{q}",
                                           tag=f"y2_{q % 2}")
                    nc.scalar.activation(
                        y2scr[:], ysl(q),
                        mybir.ActivationFunctionType.Square,
                        accum_out=py2[:, q:q + 1],
                    )
                nc.vector.reduce_sum(out=sacc[g][:, j:j + 1], in_=py[:],
                                     axis=mybir.AxisListType.X)
                nc.vector.reduce_sum(out=sacc[g][:, n + j:n + j + 1],
                                     in_=py2[:], axis=mybir.AxisListType.X)

            def do_warm(oc):
                wsrc = work_pool.tile([1, 8], F32, name=f"wsrc{oc}",
                                      tag="wsrc")
                nc.vector.memset(wsrc[:], 1.0)
                wi = dram.tile([1, 8], F32, name=f"warm_i{oc}",
                               tag=f"warm_i{oc}")
                wo = dram.tile([1, 8], F32, name=f"warm_o{oc}",
                               tag=f"warm_o{oc}")
                nc.gpsimd.dma_start(wi[:], wsrc[:])
                nc.gpsimd.collective_compute(
                    "AllReduce", mybir.AluOpType.add,
                    replica_groups=[list(range(N_CORES))],
                    ins=[wi.opt()], outs=[wo.opt()],
                )

            # ---- joint first pass: oc0 + oc1 while x streams in ----
            pts = {}
            pts[0] = psum_pool.tile([128, B_SH], F32, name="pt0", tag="pt")
            pts[1] = psum_pool.tile([128, B_SH], F32, name="pt1", tag="pt")
            for it in range(KT):
                for oc in (0, 1):
                    for bc in range(BC):
                        nc.tensor.matmul(
                            pts[oc][:, bc * NB:bc * NB + NB],
                            wbs[it][:, oc * 128:oc * 128 + 128],
                            xs[it][:, bc * NB:bc * NB + NB],
                            start=(it == 0), stop=(it == KT - 1),
                        )
            do_stats(0, pts[0])
            do_stats(1, pts[1])
            if 0 in WARM_OCS:
                do_warm(0)

            # ---- remaining passes: one oc each ----
            for oc in range(2, OC):
                pt = psum_pool.tile([128, B_SH], F32, name=f"pt{oc}",
                                    tag="pt")
                for it in range(KT):
                    for bc in range(BC):
                        nc.tensor.matmul(
                            pt[:, bc * NB:bc * NB + NB],
                            wbs[it][:, oc * 128:oc * 128 + 128],
                            xs[it][:, bc * NB:bc * NB + NB],
                            start=(it == 0), stop=(it == KT - 1),
                        )
                do_stats(oc, pt)

                if oc in WARM_OCS:
                    do_warm(oc)
                if oc == 3:
                    do_group_ar(0)       # overlaps oc4..7 matmuls
                if oc == 5:
                    do_group_coef(0)
                if oc == 6:
                    do_norm_store(0, on_vector=True)
                    do_norm_store(1, on_vector=False)
                    do_norm_store(2, on_vector=True)
                    do_norm_store(3, on_vector=True)

            # ---- tail: group B AR + its normalize/store ----
            do_group_ar(1)
            do_group_coef(1)
            do_norm_store(4, on_vector=True)
            do_norm_store(5, on_vector=False)
            do_norm_store(6, on_vector=True)
            do_norm_store(7, on_vector=True)

    nc.compile()
    return nc


_NC_CACHE = None


def kernel(x, weight, bias, gamma, beta):
    global _NC_CACHE
    if _NC_CACHE is None:
        _NC_CACHE = build_kernel()
    nc = _NC_CACHE

    import ml_dtypes
    bf16 = ml_dtypes.bfloat16

    x = np.asarray(x, dtype=np.float32)
    weight = np.asarray(weight, dtype=np.float32)
    # gamma/beta -> [128, OC] with partition p = o % 128, column oc
    gamma_t = np.ascontiguousarray(
        np.asarray(gamma, dtype=np.float32).reshape(OC, 128).T)
    beta_t = np.ascontiguousarray(
        np.asarray(beta, dtype=np.float32).reshape(OC, 128).T)

    wt = np.ascontiguousarray(weight.T).astype(bf16)
    in_maps = []
    for i in range(N_CORES):
        shard = x[i * B_SH:(i + 1) * B_SH]
        in_maps.append({
            "xt": np.ascontiguousarray(shard.T).astype(bf16),
            "wt": wt,
            "gamma": gamma_t,
            "beta": beta_t,
        })

    res = bass_utils.run_bass_kernel_spmd(
        nc, in_maps, core_ids=list(range(N_CORES)),
        trace=bool(int(os.environ.get("KERNEL_TRACE", "0"))),
    )
    kernel.last_results = res
    # device output is [o, b] bf16; transpose back and upcast
    return np.concatenate(
        [np.asarray(res.results[i]["out"]).T.astype(np.float32)
         for i in range(N_CORES)], axis=0)
